# revision 40
# baseline (speedup 1.0000x reference)
"""Trainium2 Bass kernel for nn_CGMMTransition (CGMM e-step transition posterior).

Reference math (per node n):
  neighbDim[l,a]        = sum_c2 stats[n,l,a,c2]   (0 -> 1)
  rightmost[l,a,c,c2]   = transition[l,a,c,c2] * stats[n,l,a,c2] / neighbDim[l,a]
  posterior[l,a,c,c2]   = layerS[l] * arcS[l,a] * rightmost[l,a,c,c2]
  p_Q[n,c]              = sum_{l,a,c2} posterior[n,l,a,c,c2]

Outputs dominate traffic (2 x [N,5,6,20,20] f32 = 960 MB) -> memory-bound.
Sharding: pure data parallel over N across 8 cores (1250 rows/core).

Layout: nodes on SBUF partitions; per 128-node tile the [30,20,20] block of
each node lives in the free dim.  transition (48KB) and the combined weights
layerS*arcS (120B) are replicated across partitions host-side and loaded once.
"""

import numpy as np

N, L, A, C, C2 = 10000, 5, 6, 20, 20
G = L * A                  # 30 (l,a) groups
GF = G * C2                # 600 stats elems per node
BLK = C * C2               # 400 per (l,a) output block
ROWF = G * BLK             # 12000 output elems per node per tensor
NCORES = 8
ROWS = N // NCORES         # 1250 nodes per core
P = 128                    # SBUF partitions
LA_CHUNK = 6               # (l,a) groups per output chunk (DMA granularity)
NCHUNK = G // LA_CHUNK     # 3
CHUNK_F = LA_CHUNK * BLK   # 4000

_RUN_KWARGS = {}
_LAST_RESULT = None
_NC_CACHE = {}


def _setup_path():
    import sys
    for p in ("/opt/trn_rl_repo", "/opt/trn_rl_repo/concourse"):
        if p not in sys.path:
            sys.path.insert(0, p)


def _build(rows=ROWS):
    _setup_path()
    import concourse.bacc as bacc
    import concourse.mybir as mybir
    from concourse.tile import TileContext

    f32 = mybir.dt.float32
    AX = mybir.AxisListType

    nc = bacc.Bacc("TRN2", target_bir_lowering=False, debug=False)
    stats_d = nc.dram_tensor("stats", [rows, GF], f32, kind="ExternalInput")
    trep_d = nc.dram_tensor("trep", [P, ROWF], f32, kind="ExternalInput")
    wrep_d = nc.dram_tensor("wrep", [P, G], f32, kind="ExternalInput")
    rm_d = nc.dram_tensor("rm", [rows, ROWF], f32, kind="ExternalOutput")
    post_d = nc.dram_tensor("post", [rows, ROWF], f32, kind="ExternalOutput")
    pq_d = nc.dram_tensor("pq", [rows, C], f32, kind="ExternalOutput")

    with TileContext(nc) as tc:
        with (
            tc.tile_pool(name="const", bufs=1) as cpool,
            tc.tile_pool(name="small", bufs=2) as spool,
            tc.tile_pool(name="rmp", bufs=6) as rmpool,
            tc.tile_pool(name="postp", bufs=6) as postpool,
        ):
            # const loads on the GpSimd SWDGE queue so they don't serialize
            # ahead of the first stats tiles on the Sync queue
            trep = cpool.tile([P, ROWF], f32)
            nc.gpsimd.dma_start(out=trep[:], in_=trep_d[:, :])
            wrep = cpool.tile([P, G], f32)
            nc.gpsimd.dma_start(out=wrep[:], in_=wrep_d[:, :])

            ntiles = (rows + P - 1) // P
            for t in range(ntiles):
                r0 = t * P
                h = min(P, rows - r0)

                stats = spool.tile([P, GF], f32, tag="stats")
                nc.sync.dma_start(out=stats[:h], in_=stats_d[r0 : r0 + h])
                sview = stats[:h].rearrange("p (g d) -> p g d", g=G, d=C2)

                # neighbDim + zero->1 fix + reciprocal
                neighb = spool.tile([P, G], f32, tag="neighb")
                nc.vector.reduce_sum(out=neighb[:h], in_=sview, axis=AX.X)
                # fix zeros: nb + (nb == 0) in one op
                nbfix = spool.tile([P, G], f32, tag="nbfix")
                nc.vector.scalar_tensor_tensor(
                    out=nbfix[:h], in0=neighb[:h], scalar=0.0, in1=neighb[:h],
                    op0=mybir.AluOpType.is_equal, op1=mybir.AluOpType.add,
                )
                recip = spool.tile([P, G], f32, tag="recip")
                nc.vector.reciprocal(out=recip[:h], in_=nbfix[:h])

                # scaled[n,g,c2] = stats * (1/neighb) broadcast over c2
                scaled = spool.tile([P, GF], f32, tag="scaled")
                scview = scaled[:h].rearrange("p (g d) -> p g d", g=G, d=C2)
                nc.vector.tensor_mul(
                    out=scview,
                    in0=sview,
                    in1=recip[:h, :, None].broadcast_to([h, G, C2]),
                )

                pq_parts = spool.tile([P, NCHUNK, C], f32, tag="pqp")
                for k in range(NCHUNK):
                    # rightmost[n,g,c,c2] = trep[g,c,c2] * scaled[n,g,c2]
                    rm = rmpool.tile([P, CHUNK_F], f32, tag="rm")
                    rmv = rm[:h].rearrange(
                        "p (g c d) -> p g c d", g=LA_CHUNK, c=C, d=C2
                    )
                    tv = trep[:h, k * CHUNK_F : (k + 1) * CHUNK_F].rearrange(
                        "p (g c d) -> p g c d", g=LA_CHUNK, c=C, d=C2
                    )
                    sb = scview[:, k * LA_CHUNK : (k + 1) * LA_CHUNK, None, :]
                    nc.vector.tensor_mul(
                        out=rmv, in0=tv,
                        in1=sb.broadcast_to([h, LA_CHUNK, C, C2]),
                    )

                    # posterior = w[la] * rightmost on ACT
                    post = postpool.tile([P, CHUNK_F], f32, tag="post")
                    for j in range(LA_CHUNK):
                        la = k * LA_CHUNK + j
                        o = post[:h, j * BLK : (j + 1) * BLK]
                        i_ = rm[:h, j * BLK : (j + 1) * BLK]
                        nc.scalar.mul(o, i_, wrep[:h, la : la + 1])

                    # p_Q partial: sum posterior over (g, c2), keep c
                    pv = post[:h].rearrange(
                        "p (g c d) -> p c g d", g=LA_CHUNK, c=C, d=C2
                    )
                    nc.vector.reduce_sum(out=pq_parts[:h, k, :], in_=pv, axis=AX.XY)

                    nc.sync.dma_start(
                        out=rm_d[r0 : r0 + h, k * CHUNK_F : (k + 1) * CHUNK_F],
                        in_=rm[:h],
                    )
                    nc.sync.dma_start(
                        out=post_d[r0 : r0 + h, k * CHUNK_F : (k + 1) * CHUNK_F],
                        in_=post[:h],
                    )

                pq = spool.tile([P, C], f32, tag="pq")
                nc.vector.reduce_sum(
                    out=pq[:h],
                    in_=pq_parts[:h].rearrange("p k c -> p c k"),
                    axis=AX.X,
                )
                nc.sync.dma_start(out=pq_d[r0 : r0 + h], in_=pq[:h])

    nc.compile()
    return nc


def _host_inputs(stats, layerS, arcS, transition):
    stats = np.ascontiguousarray(np.asarray(stats, dtype=np.float32).reshape(N, GF))
    w = (
        np.asarray(layerS, dtype=np.float32)[:, None]
        * np.asarray(arcS, dtype=np.float32)
    ).reshape(G)
    trep = np.ascontiguousarray(
        np.broadcast_to(
            np.asarray(transition, dtype=np.float32).reshape(1, ROWF), (P, ROWF)
        )
    )
    wrep = np.ascontiguousarray(np.broadcast_to(w.reshape(1, G), (P, G)))
    return stats, trep, wrep


def kernel(stats, layerS, arcS, transition):
    global _LAST_RESULT
    _setup_path()
    from concourse.bass_utils import run_bass_kernel_spmd

    stats2, trep, wrep = _host_inputs(stats, layerS, arcS, transition)

    if "nc" not in _NC_CACHE:
        _NC_CACHE["nc"] = _build()
    nc = _NC_CACHE["nc"]

    in_maps = [
        {
            "stats": np.ascontiguousarray(stats2[i * ROWS : (i + 1) * ROWS]),
            "trep": trep,
            "wrep": wrep,
        }
        for i in range(NCORES)
    ]
    res = run_bass_kernel_spmd(
        nc, in_maps, core_ids=list(range(NCORES)), **_RUN_KWARGS
    )
    _LAST_RESULT = res
    rs = res.results

    pq = np.concatenate([r["pq"] for r in rs], axis=0)
    post = np.concatenate([r["post"] for r in rs], axis=0).reshape(N, L, A, C, C2)
    rm = np.concatenate([r["rm"] for r in rs], axis=0).reshape(N, L, A, C, C2)
    return pq, post, rm


# revision 41
# speedup vs baseline: 1.0125x; 1.0125x over previous
"""Trainium2 Bass kernel for nn_CGMMTransition (CGMM e-step transition posterior).

Reference math (per node n):
  neighbDim[l,a]        = sum_c2 stats[n,l,a,c2]   (0 -> 1)
  rightmost[l,a,c,c2]   = transition[l,a,c,c2] * stats[n,l,a,c2] / neighbDim[l,a]
  posterior[l,a,c,c2]   = layerS[l] * arcS[l,a] * rightmost[l,a,c,c2]
  p_Q[n,c]              = sum_{l,a,c2} posterior[n,l,a,c,c2]

Outputs dominate traffic (2 x [N,5,6,20,20] f32 = 960 MB) -> memory-bound.
Sharding: pure data parallel over N across 8 cores (1250 rows/core).

Layout: nodes on SBUF partitions; per 128-node tile the [30,20,20] block of
each node lives in the free dim.  transition (48KB) and the combined weights
layerS*arcS (120B) are replicated across partitions host-side and loaded once.
"""

import numpy as np

N, L, A, C, C2 = 10000, 5, 6, 20, 20
G = L * A                  # 30 (l,a) groups
GF = G * C2                # 600 stats elems per node
BLK = C * C2               # 400 per (l,a) output block
ROWF = G * BLK             # 12000 output elems per node per tensor
NCORES = 8
ROWS = N // NCORES         # 1250 nodes per core
P = 128                    # SBUF partitions
LA_CHUNK = 6               # (l,a) groups per output chunk (DMA granularity)
NCHUNK = G // LA_CHUNK     # 3
CHUNK_F = LA_CHUNK * BLK   # 4000

_RUN_KWARGS = {}
_LAST_RESULT = None
_NC_CACHE = {}


def _setup_path():
    import sys
    for p in ("/opt/trn_rl_repo", "/opt/trn_rl_repo/concourse"):
        if p not in sys.path:
            sys.path.insert(0, p)


def _build(rows=ROWS):
    _setup_path()
    import concourse.bacc as bacc
    import concourse.mybir as mybir
    from concourse.tile import TileContext

    f32 = mybir.dt.float32
    AX = mybir.AxisListType

    nc = bacc.Bacc("TRN2", target_bir_lowering=False, debug=False)
    stats_d = nc.dram_tensor("stats", [rows, GF], f32, kind="ExternalInput")
    trep_d = nc.dram_tensor("trep", [P, ROWF], f32, kind="ExternalInput")
    wrep_d = nc.dram_tensor("wrep", [P, G], f32, kind="ExternalInput")
    rm_d = nc.dram_tensor("rm", [rows, ROWF], f32, kind="ExternalOutput")
    post_d = nc.dram_tensor("post", [rows, ROWF], f32, kind="ExternalOutput")
    pq_d = nc.dram_tensor("pq", [rows, C], f32, kind="ExternalOutput")

    with TileContext(nc) as tc:
        with (
            tc.tile_pool(name="const", bufs=1) as cpool,
            tc.tile_pool(name="small", bufs=2) as spool,
            tc.tile_pool(name="rmp", bufs=5) as rmpool,
            tc.tile_pool(name="postp", bufs=5) as postpool,
        ):
            # const loads on the GpSimd SWDGE queue so they don't serialize
            # ahead of the first stats tiles on the Sync queue; trep split
            # per chunk so the first rm gates on 1.2MB, not 6MB
            wrep = cpool.tile([P, G], f32)
            nc.gpsimd.dma_start(out=wrep[:], in_=wrep_d[:, :])
            treps = []
            for k in range(NCHUNK):
                tk = cpool.tile([P, CHUNK_F], f32, tag=f"trep{k}")
                nc.gpsimd.dma_start(
                    out=tk[:], in_=trep_d[:, k * CHUNK_F : (k + 1) * CHUNK_F]
                )
                treps.append(tk)

            ntiles = (rows + P - 1) // P
            for t in range(ntiles):
                r0 = t * P
                h = min(P, rows - r0)

                stats = spool.tile([P, GF], f32, tag="stats")
                nc.sync.dma_start(out=stats[:h], in_=stats_d[r0 : r0 + h])
                sview = stats[:h].rearrange("p (g d) -> p g d", g=G, d=C2)

                # neighbDim + zero->1 fix + reciprocal
                neighb = spool.tile([P, G], f32, tag="neighb")
                nc.vector.reduce_sum(out=neighb[:h], in_=sview, axis=AX.X)
                # fix zeros: nb + (nb == 0) in one op
                nbfix = spool.tile([P, G], f32, tag="nbfix")
                nc.vector.scalar_tensor_tensor(
                    out=nbfix[:h], in0=neighb[:h], scalar=0.0, in1=neighb[:h],
                    op0=mybir.AluOpType.is_equal, op1=mybir.AluOpType.add,
                )
                recip = spool.tile([P, G], f32, tag="recip")
                nc.vector.reciprocal(out=recip[:h], in_=nbfix[:h])

                # scaled[n,g,c2] = stats * (1/neighb) broadcast over c2
                scaled = spool.tile([P, GF], f32, tag="scaled")
                scview = scaled[:h].rearrange("p (g d) -> p g d", g=G, d=C2)
                nc.vector.tensor_mul(
                    out=scview,
                    in0=sview,
                    in1=recip[:h, :, None].broadcast_to([h, G, C2]),
                )

                pq_parts = spool.tile([P, NCHUNK, C], f32, tag="pqp")
                for k in range(NCHUNK):
                    # rightmost[n,g,c,c2] = trep[g,c,c2] * scaled[n,g,c2]
                    rm = rmpool.tile([P, CHUNK_F], f32, tag="rm")
                    rmv = rm[:h].rearrange(
                        "p (g c d) -> p g c d", g=LA_CHUNK, c=C, d=C2
                    )
                    tv = treps[k][:h].rearrange(
                        "p (g c d) -> p g c d", g=LA_CHUNK, c=C, d=C2
                    )
                    sb = scview[:, k * LA_CHUNK : (k + 1) * LA_CHUNK, None, :]
                    nc.vector.tensor_mul(
                        out=rmv, in0=tv,
                        in1=sb.broadcast_to([h, LA_CHUNK, C, C2]),
                    )

                    # posterior = w[la] * rightmost on ACT
                    post = postpool.tile([P, CHUNK_F], f32, tag="post")
                    for j in range(LA_CHUNK):
                        la = k * LA_CHUNK + j
                        o = post[:h, j * BLK : (j + 1) * BLK]
                        i_ = rm[:h, j * BLK : (j + 1) * BLK]
                        nc.scalar.mul(o, i_, wrep[:h, la : la + 1])

                    # p_Q partial: sum posterior over (g, c2), keep c
                    pv = post[:h].rearrange(
                        "p (g c d) -> p c g d", g=LA_CHUNK, c=C, d=C2
                    )
                    nc.vector.reduce_sum(out=pq_parts[:h, k, :], in_=pv, axis=AX.XY)

                    nc.sync.dma_start(
                        out=rm_d[r0 : r0 + h, k * CHUNK_F : (k + 1) * CHUNK_F],
                        in_=rm[:h],
                    )
                    nc.sync.dma_start(
                        out=post_d[r0 : r0 + h, k * CHUNK_F : (k + 1) * CHUNK_F],
                        in_=post[:h],
                    )

                pq = spool.tile([P, C], f32, tag="pq")
                nc.vector.reduce_sum(
                    out=pq[:h],
                    in_=pq_parts[:h].rearrange("p k c -> p c k"),
                    axis=AX.X,
                )
                nc.sync.dma_start(out=pq_d[r0 : r0 + h], in_=pq[:h])

    nc.compile()
    return nc


def _host_inputs(stats, layerS, arcS, transition):
    stats = np.ascontiguousarray(np.asarray(stats, dtype=np.float32).reshape(N, GF))
    w = (
        np.asarray(layerS, dtype=np.float32)[:, None]
        * np.asarray(arcS, dtype=np.float32)
    ).reshape(G)
    trep = np.ascontiguousarray(
        np.broadcast_to(
            np.asarray(transition, dtype=np.float32).reshape(1, ROWF), (P, ROWF)
        )
    )
    wrep = np.ascontiguousarray(np.broadcast_to(w.reshape(1, G), (P, G)))
    return stats, trep, wrep


def kernel(stats, layerS, arcS, transition):
    global _LAST_RESULT
    _setup_path()
    from concourse.bass_utils import run_bass_kernel_spmd

    stats2, trep, wrep = _host_inputs(stats, layerS, arcS, transition)

    if "nc" not in _NC_CACHE:
        _NC_CACHE["nc"] = _build()
    nc = _NC_CACHE["nc"]

    in_maps = [
        {
            "stats": np.ascontiguousarray(stats2[i * ROWS : (i + 1) * ROWS]),
            "trep": trep,
            "wrep": wrep,
        }
        for i in range(NCORES)
    ]
    res = run_bass_kernel_spmd(
        nc, in_maps, core_ids=list(range(NCORES)), **_RUN_KWARGS
    )
    _LAST_RESULT = res
    rs = res.results

    pq = np.concatenate([r["pq"] for r in rs], axis=0)
    post = np.concatenate([r["post"] for r in rs], axis=0).reshape(N, L, A, C, C2)
    rm = np.concatenate([r["rm"] for r in rs], axis=0).reshape(N, L, A, C, C2)
    return pq, post, rm


# revision 42
# speedup vs baseline: 1.1133x; 1.0995x over previous
"""Trainium2 Bass kernel for nn_CGMMTransition (CGMM e-step transition posterior).

Reference math (per node n):
  neighbDim[l,a]        = sum_c2 stats[n,l,a,c2]   (0 -> 1)
  rightmost[l,a,c,c2]   = transition[l,a,c,c2] * stats[n,l,a,c2] / neighbDim[l,a]
  posterior[l,a,c,c2]   = layerS[l] * arcS[l,a] * rightmost[l,a,c,c2]
  p_Q[n,c]              = sum_{l,a,c2} posterior[n,l,a,c,c2]

Outputs dominate traffic (2 x [N,5,6,20,20] f32 = 960 MB) -> memory-bound.
Sharding: pure data parallel over N across 8 cores (1250 rows/core).

Layout: nodes on SBUF partitions; per 128-node tile the [30,20,20] block of
each node lives in the free dim.  transition (48KB) and the combined weights
layerS*arcS (120B) are replicated across partitions host-side and loaded once.
"""

import numpy as np

N, L, A, C, C2 = 10000, 5, 6, 20, 20
G = L * A                  # 30 (l,a) groups
GF = G * C2                # 600 stats elems per node
BLK = C * C2               # 400 per (l,a) output block
ROWF = G * BLK             # 12000 output elems per node per tensor
NCORES = 8
ROWS = N // NCORES         # 1250 nodes per core
P = 128                    # SBUF partitions
LA_CHUNK = 6               # (l,a) groups per output chunk (DMA granularity)
NCHUNK = G // LA_CHUNK     # 3
CHUNK_F = LA_CHUNK * BLK   # 4000

_RUN_KWARGS = {}
_LAST_RESULT = None
_NC_CACHE = {}


def _setup_path():
    import sys
    for p in ("/opt/trn_rl_repo", "/opt/trn_rl_repo/concourse"):
        if p not in sys.path:
            sys.path.insert(0, p)


def _build(rows=ROWS):
    _setup_path()
    import concourse.bacc as bacc
    import concourse.mybir as mybir
    from concourse.tile import TileContext

    f32 = mybir.dt.float32
    AX = mybir.AxisListType

    nc = bacc.Bacc("TRN2", target_bir_lowering=False, debug=False)
    stats_d = nc.dram_tensor("stats", [rows, GF], f32, kind="ExternalInput")
    trep_d = nc.dram_tensor("trep", [P, ROWF], f32, kind="ExternalInput")
    wrep_d = nc.dram_tensor("wrep", [P, G], f32, kind="ExternalInput")
    rm_d = nc.dram_tensor("rm", [rows, ROWF], f32, kind="ExternalOutput")
    post_d = nc.dram_tensor("post", [rows, ROWF], f32, kind="ExternalOutput")
    pq_d = nc.dram_tensor("pq", [rows, C], f32, kind="ExternalOutput")

    with TileContext(nc) as tc:
        with (
            tc.tile_pool(name="const", bufs=1) as cpool,
            tc.tile_pool(name="small", bufs=2) as spool,
            tc.tile_pool(name="rmp", bufs=5) as rmpool,
            tc.tile_pool(name="postp", bufs=5) as postpool,
        ):
            # const loads on the GpSimd SWDGE queue so they don't serialize
            # ahead of the first stats tiles on the Sync queue
            trep = cpool.tile([P, ROWF], f32)
            nc.gpsimd.dma_start(out=trep[:], in_=trep_d[:, :])
            wrep = cpool.tile([P, G], f32)
            nc.gpsimd.dma_start(out=wrep[:], in_=wrep_d[:, :])

            ntiles = (rows + P - 1) // P
            for t in range(ntiles):
                r0 = t * P
                h = min(P, rows - r0)

                stats = spool.tile([P, GF], f32, tag="stats")
                nc.sync.dma_start(out=stats[:h], in_=stats_d[r0 : r0 + h])
                sview = stats[:h].rearrange("p (g d) -> p g d", g=G, d=C2)

                # neighbDim + zero->1 fix + reciprocal
                neighb = spool.tile([P, G], f32, tag="neighb")
                nc.vector.reduce_sum(out=neighb[:h], in_=sview, axis=AX.X)
                # fix zeros: nb + (nb == 0) in one op
                nbfix = spool.tile([P, G], f32, tag="nbfix")
                nc.vector.scalar_tensor_tensor(
                    out=nbfix[:h], in0=neighb[:h], scalar=0.0, in1=neighb[:h],
                    op0=mybir.AluOpType.is_equal, op1=mybir.AluOpType.add,
                )
                recip = spool.tile([P, G], f32, tag="recip")
                nc.vector.reciprocal(out=recip[:h], in_=nbfix[:h])

                # scaled[n,g,c2] = stats * (1/neighb) broadcast over c2
                scaled = spool.tile([P, GF], f32, tag="scaled")
                scview = scaled[:h].rearrange("p (g d) -> p g d", g=G, d=C2)
                nc.vector.tensor_mul(
                    out=scview,
                    in0=sview,
                    in1=recip[:h, :, None].broadcast_to([h, G, C2]),
                )

                pq_parts = spool.tile([P, NCHUNK, C], f32, tag="pqp")
                for k in range(NCHUNK):
                    # rightmost[n,g,c,c2] = trep[g,c,c2] * scaled[n,g,c2]
                    rm = rmpool.tile([P, CHUNK_F], f32, tag="rm")
                    rmv = rm[:h].rearrange(
                        "p (g c d) -> p g c d", g=LA_CHUNK, c=C, d=C2
                    )
                    tv = trep[:h, k * CHUNK_F : (k + 1) * CHUNK_F].rearrange(
                        "p (g c d) -> p g c d", g=LA_CHUNK, c=C, d=C2
                    )
                    sb = scview[:, k * LA_CHUNK : (k + 1) * LA_CHUNK, None, :]
                    nc.vector.tensor_mul(
                        out=rmv, in0=tv,
                        in1=sb.broadcast_to([h, LA_CHUNK, C, C2]),
                    )

                    # posterior = w[la] * rightmost on ACT
                    post = postpool.tile([P, CHUNK_F], f32, tag="post")
                    for j in range(LA_CHUNK):
                        la = k * LA_CHUNK + j
                        o = post[:h, j * BLK : (j + 1) * BLK]
                        i_ = rm[:h, j * BLK : (j + 1) * BLK]
                        nc.scalar.mul(o, i_, wrep[:h, la : la + 1])

                    # p_Q partial: sum posterior over (g, c2), keep c
                    pv = post[:h].rearrange(
                        "p (g c d) -> p c g d", g=LA_CHUNK, c=C, d=C2
                    )
                    nc.vector.reduce_sum(out=pq_parts[:h, k, :], in_=pv, axis=AX.XY)

                    nc.sync.dma_start(
                        out=rm_d[r0 : r0 + h, k * CHUNK_F : (k + 1) * CHUNK_F],
                        in_=rm[:h],
                    )
                    nc.sync.dma_start(
                        out=post_d[r0 : r0 + h, k * CHUNK_F : (k + 1) * CHUNK_F],
                        in_=post[:h],
                    )

                pq = spool.tile([P, C], f32, tag="pq")
                nc.vector.reduce_sum(
                    out=pq[:h],
                    in_=pq_parts[:h].rearrange("p k c -> p c k"),
                    axis=AX.X,
                )
                nc.sync.dma_start(out=pq_d[r0 : r0 + h], in_=pq[:h])

    nc.compile()
    return nc


def _host_inputs(stats, layerS, arcS, transition):
    stats = np.ascontiguousarray(np.asarray(stats, dtype=np.float32).reshape(N, GF))
    w = (
        np.asarray(layerS, dtype=np.float32)[:, None]
        * np.asarray(arcS, dtype=np.float32)
    ).reshape(G)
    trep = np.ascontiguousarray(
        np.broadcast_to(
            np.asarray(transition, dtype=np.float32).reshape(1, ROWF), (P, ROWF)
        )
    )
    wrep = np.ascontiguousarray(np.broadcast_to(w.reshape(1, G), (P, G)))
    return stats, trep, wrep


def kernel(stats, layerS, arcS, transition):
    global _LAST_RESULT
    _setup_path()
    from concourse.bass_utils import run_bass_kernel_spmd

    stats2, trep, wrep = _host_inputs(stats, layerS, arcS, transition)

    if "nc" not in _NC_CACHE:
        _NC_CACHE["nc"] = _build()
    nc = _NC_CACHE["nc"]

    in_maps = [
        {
            "stats": np.ascontiguousarray(stats2[i * ROWS : (i + 1) * ROWS]),
            "trep": trep,
            "wrep": wrep,
        }
        for i in range(NCORES)
    ]
    res = run_bass_kernel_spmd(
        nc, in_maps, core_ids=list(range(NCORES)), **_RUN_KWARGS
    )
    _LAST_RESULT = res
    rs = res.results

    pq = np.concatenate([r["pq"] for r in rs], axis=0)
    post = np.concatenate([r["post"] for r in rs], axis=0).reshape(N, L, A, C, C2)
    rm = np.concatenate([r["rm"] for r in rs], axis=0).reshape(N, L, A, C, C2)
    return pq, post, rm
